# revision 19
# baseline (speedup 1.0000x reference)
"""Trainium2 Bass kernel: teacher-forced LSTM decoder + packed vocab projection.

Model (B=128, T=20, E=H=512, V=32000):
  x = [features, embed(captions[:, :T-1])]            # [B, T, E]
  (h, c) LSTM-scan over T steps (PyTorch gate order i,f,g,o)
  logits = hs @ lin_w.T + lin_b                       # [T, B, V]
  out = logits packed time-major, keeping rows with length > t  # [sum(len), V]

Strategy (8 NeuronCores, v3):
  - Vocab-parallel: core s owns lin_w columns [s*4000, (s+1)*4000).
  - The x-part of the gates (x @ w_ih.T + bias) is computed on the HOST:
    teacher-forced inputs are embedding rows, so it is a row lookup into
    embed_w[used_tokens] @ w_ih.T (small BLAS) plus features @ w_ih.T.
    The device receives xWT [4H, L] bf16 and spends its PE only on the
    recurrent h-part and the output projection.
  - Everything on device is in TRANSPOSED orientation (features on
    partitions), all matmuls bf16: gate matmuls stream only n_t columns per
    step, h comes out of the LSTM already as hsT [H, L] (no PE transposes).
  - Per step t: 64 matmuls accumulate whh.T-chunks @ hT_{t-1} into 4 PSUM
    banks [128, 4, n] (bank j = gate j); DVE adds the xWT slice; ScalarE
    sigmoid/tanh; DVE c/h update writes h directly into the bf16 hsT stash
    (which doubles as next step's matmul moving operand and the projection's
    stationary operand). Step 0 with zero h0/c0 needs no matmuls at all.
  - The PE queue is a hand-scheduled FIFO: projection units are interleaved
    between recurrence steps as fillers so the PE never idles during the
    serial per-step activation chain.
  - Projection: for each 128-row L-chunk x 500-wide vocab slice, 4 bf16
    matmuls (hsT chunk stationary, linT moving) -> PSUM -> SBUF; output DMAs
    are issued per 1000-column pair to halve the Sync-engine descriptor load.
  - Host gathers the 8 core outputs and concatenates along vocab.
"""

import math
from collections import deque

import numpy as np
import ml_dtypes

import concourse.bacc as bacc
import concourse.bass as bass
import concourse.mybir as mybir
import concourse.tile as tile
from concourse.bass_utils import run_bass_kernel_spmd

B, T, E, H, V = 128, 20, 512, 512, 32000
NCORES = 8
VS = V // NCORES      # per-core vocab shard (4000)
NV = 8                # vocab sub-chunks per core
VC = VS // NV         # 500 columns per projection matmul
KE = E // 128
KH = H // 128
P = 128
G4 = 4 * H            # 2048 gate dims
NM = G4 // P          # 16 gate M-chunks

F32 = mybir.dt.float32
BF16 = mybir.dt.bfloat16
AF = mybir.ActivationFunctionType

BF = ml_dtypes.bfloat16

FILL_NS = 3400        # PE filler budget per recurrence gap (hides act chain)


def build_program(n_ts, zero_init, use_linb):
    """Single-core Bass/Tile program (same program on all 8 cores)."""
    L = int(sum(n_ts))
    offs = np.concatenate([[0], np.cumsum(n_ts)]).astype(int)
    nchunks = math.ceil(L / P)

    nc = bacc.Bacc("TRN2", target_bir_lowering=False, debug=False)

    xWT_d = nc.dram_tensor("xWT", [G4, L], BF16, kind="ExternalInput")
    # step-0 slice of xWT, host-pre-arranged [p, m*n0+l] so the head DMA is
    # one dense descriptor (the [G4, L] layout only yields 256B packets)
    n0 = int(n_ts[0])
    xWh_d = nc.dram_tensor("xWh", [P, NM * n0], BF16, kind="ExternalInput")
    whh_d = nc.dram_tensor("whh", [H, G4], BF16, kind="ExternalInput")
    linT_d = nc.dram_tensor("linT", [H, VS], BF16, kind="ExternalInput")
    h0T_d = c0T_d = None
    if not zero_init:
        h0T_d = nc.dram_tensor("h0T", [H, B], BF16, kind="ExternalInput")
        c0T_d = nc.dram_tensor("c0T", [H, B], F32, kind="ExternalInput")
    linb_d = None
    if use_linb:
        linb_d = nc.dram_tensor("linb", [1, VS], BF16, kind="ExternalInput")
    # bf16 output (host upcasts): halves the output DMA, and the 8 cores
    # together sit near the aggregate HBM roofline
    out_d = nc.dram_tensor("out", [L, VS], BF16, kind="ExternalOutput")

    PS = bass.MemorySpace.PSUM

    with tile.TileContext(nc) as tc:
        with (
            tc.tile_pool(name="persist", bufs=1) as pers,
            tc.tile_pool(name="cc", bufs=2) as ccp,
            tc.tile_pool(name="work", bufs=12) as wkp,
            tc.tile_pool(name="outs", bufs=4) as otp,
            tc.tile_pool(name="gps", bufs=4, space=PS) as gpsp,
            tc.tile_pool(name="pps", bufs=4, space=PS) as ppsp,
        ):
            whh_sb = pers.tile([P, KH, NM, P], BF16, tag="whh")
            lin_sb = pers.tile([P, KH, VS], BF16, tag="lin")
            xWT = pers.tile([P, NM, L], BF16, tag="xWT")
            hsT = pers.tile([P, KH, L], BF16, tag="hsT")

            # DMA plan: few wide descriptors in need order, issue split across
            # the Sync queue (xWT/whh/lin-a) and the otherwise-idle GpSimd
            # queue (lin-b + all output DMAs) so descriptor issue (~0.65us
            # each) doesn't serialize the head.
            mid = min(n0 + 512, L)
            nc.sync.dma_start(
                xWT[:, :, :n0], xWh_d[:].rearrange("p (m l) -> p m l", m=NM)
            )
            h0_sb = c0_sb = None
            if not zero_init:
                h0_sb = pers.tile([P, KH, P], BF16, tag="h0")
                nc.sync.dma_start(h0_sb[:], h0T_d[:].rearrange("(k p) b -> p k b", k=KH))
                c0_sb = ccp.tile([P, KH, P], F32, tag="c", name="c0t")
                nc.sync.dma_start(c0_sb[:], c0T_d[:].rearrange("(k p) b -> p k b", k=KH))
            for k in range(KH):
                nc.sync.dma_start(lin_sb[:, k, : 4 * VC], linT_d[P * k : P * (k + 1), : 4 * VC])
                nc.sync.dma_start(whh_sb[:, k, :, :], whh_d[P * k : P * (k + 1), :])
            # issue the bulk xWT transfers from the Scalar queue (idle at the
            # head) so they start ~8us earlier than Sync could issue them
            if mid > n0:
                for hf in range(2):
                    src = xWT_d[1024 * hf : 1024 * (hf + 1), n0:mid]
                    nc.scalar.dma_start(xWT[:, 8 * hf : 8 * (hf + 1), n0:mid],
                                        src.rearrange("(m p) l -> p m l", m=8))
            if L > mid:
                for hf in range(2):
                    src = xWT_d[1024 * hf : 1024 * (hf + 1), mid:]
                    nc.scalar.dma_start(xWT[:, 8 * hf : 8 * (hf + 1), mid:],
                                        src.rearrange("(m p) l -> p m l", m=8))
            for k in range(KH):
                nc.gpsimd.dma_start(lin_sb[:, k, 4 * VC :], linT_d[P * k : P * (k + 1), 4 * VC :])
            ones_t = linb_sb = None
            if use_linb:
                ones_t = pers.tile([1, P], BF16, tag="ones")
                nc.vector.memset(ones_t[:], 1.0)
                linb_sb = pers.tile([1, VS], BF16, tag="linb")
                nc.gpsimd.dma_start(linb_sb[:], linb_d[:])

            # PE pre-warm during the head DMA window: ramps the pstate clock
            # with junk matmuls on a memset tile (results never read)
            junk = pers.tile([P, 512], BF16, tag="junk")
            nc.vector.memset(junk[:], 0.5)
            for w in range(12):
                wp = ppsp.tile([P, VC], F32, tag="pp", name="warm")
                nc.tensor.matmul(wp[:], junk[:, :128], junk[:, :VC], start=True, stop=True)

            # ---- projection units (PE fillers between recurrence steps) ----
            ot_state = {}

            def emit_proj(c, v, alt):
                mc = min(P, L - P * c)
                vs = slice(VC * v, VC * (v + 1))
                pp = ppsp.tile([P, VC], F32, tag="pp", name="pp")
                for k in range(KH):
                    nc.tensor.matmul(
                        pp[:mc, :], hsT[:, k, P * c : P * c + mc], lin_sb[:, k, vs],
                        start=(k == 0), stop=(k == KH - 1) and not use_linb,
                    )
                if use_linb:
                    nc.tensor.matmul(
                        pp[:mc, :], ones_t[:1, :mc], linb_sb[:1, vs],
                        start=False, stop=True,
                    )
                # stage pairs (v even, v odd) into one [128, 1000] tile: one
                # output DMA per pair halves the Sync descriptor load
                if v % 2 == 0:
                    ot = otp.tile([P, 2 * VC], BF16, tag="ot", name="ot")
                    ot_state[c] = ot
                else:
                    ot = ot_state.pop(c)
                half = slice((v % 2) * VC, (v % 2) * VC + VC)
                if alt % 2 == 0:
                    nc.vector.tensor_copy(ot[:mc, half], pp[:mc, :])
                else:
                    nc.scalar.copy(ot[:mc, half], pp[:mc, :])
                if v % 2 == 1:
                    eng = nc.gpsimd if (c + v) % 4 < 2 else nc.sync
                    eng.dma_start(
                        out_d[P * c : P * c + mc, VC * (v - 1) : VC * (v + 1)],
                        ot[:mc, :],
                    )

            MM = 0.52  # ns per moving column
            fillers = deque()
            proj_added = 0
            n_proj = 0

            def add_ready_projs(done_rows):
                nonlocal proj_added, n_proj
                while (proj_added + 1) * P <= done_rows or (
                    proj_added == nchunks - 1 and done_rows >= L
                ):
                    c = proj_added
                    for v in range(NV):
                        cost = KH * max(41.0, MM * VC)
                        fillers.append((cost, emit_proj, (c, v, n_proj)))
                        n_proj += 1
                    proj_added += 1

            def fill(budget_ns):
                spent = 0.0
                while fillers and spent < budget_ns:
                    cost, fn, args = fillers.popleft()
                    fn(*args)
                    spent += cost

            # ---- recurrence over packed steps ----
            c_prev = c0_sb
            for t, n in enumerate(n_ts):
                n = int(n)
                off = int(offs[t])
                skip_mm = t == 0 and zero_init
                if not skip_mm:
                    if t == 0:
                        hT = h0_sb
                        hsl = slice(0, n)
                    else:
                        hT = hsT
                        po = int(offs[t - 1])
                        hsl = slice(po, po + n)
                    banks = []
                    for j in range(4):
                        g = gpsp.tile([P, 4, P], F32, tag="g", name="g")
                        for m in range(4):
                            for k in range(KH):
                                nc.tensor.matmul(
                                    g[:, m, :n],
                                    whh_sb[:, k, 4 * j + m, :],
                                    hT[:, k, hsl],
                                    start=(k == 0), stop=(k == KH - 1),
                                )
                        banks.append(g)

                # gates = xWT (+ h-part); nonlinearities; c/h update
                acts = []
                for j, af in enumerate((AF.Sigmoid, AF.Sigmoid, AF.Tanh, AF.Sigmoid)):
                    xsl = xWT[:, 4 * j : 4 * j + 4, off : off + n]
                    a = wkp.tile([P, 4, P], F32, tag="wk")
                    if skip_mm:
                        nc.scalar.activation(a[:, :, :n], xsl, af)
                    else:
                        tmp = wkp.tile([P, 4, P], F32, tag="wk")
                        nc.vector.tensor_add(tmp[:, :, :n], banks[j][:, :, :n], xsl)
                        nc.scalar.activation(a[:, :, :n], tmp[:, :, :n], af)
                    acts.append(a)
                i_s, f_s, g_s, o_s = acts
                c_new = ccp.tile([P, KH, P], F32, tag="c", name="cn")
                if skip_mm and c_prev is None:
                    nc.vector.tensor_mul(c_new[:, :, :n], i_s[:, :, :n], g_s[:, :, :n])
                else:
                    t1 = wkp.tile([P, 4, P], F32, tag="wk")
                    t2 = wkp.tile([P, 4, P], F32, tag="wk")
                    nc.vector.tensor_mul(t1[:, :, :n], i_s[:, :, :n], g_s[:, :, :n])
                    nc.vector.tensor_mul(t2[:, :, :n], f_s[:, :, :n], c_prev[:, :, :n])
                    nc.vector.tensor_add(c_new[:, :, :n], t1[:, :, :n], t2[:, :, :n])
                tct = wkp.tile([P, 4, P], F32, tag="wk")
                nc.scalar.activation(tct[:, :, :n], c_new[:, :, :n], AF.Tanh)
                nc.vector.tensor_mul(
                    hsT[:, :, off : off + n], o_s[:, :, :n], tct[:, :, :n]
                )
                c_prev = c_new

                add_ready_projs(int(offs[t + 1]))
                if t + 1 < len(n_ts):
                    fill(FILL_NS)

            # ---- tail: flush remaining projection units ----
            add_ready_projs(L)
            while fillers:
                cost, fn, args = fillers.popleft()
                fn(*args)

    nc.compile()
    return nc


_prog_cache = {}


def _get_program(n_ts, zero_init, use_linb):
    key = (tuple(int(x) for x in n_ts), bool(zero_init), bool(use_linb))
    if key not in _prog_cache:
        _prog_cache[key] = build_program(n_ts, zero_init, use_linb)
    return _prog_cache[key]


def kernel(
    features,
    captions,
    lengths,
    h0,
    c0,
    embed_w,
    w_ih,
    w_hh,
    b_ih,
    b_hh,
    lin_w,
    lin_b,
    maxlen,
    _trace=False,
):
    features = np.asarray(features, np.float32)
    captions = np.asarray(captions)
    lengths = np.asarray(lengths)
    h0 = np.asarray(h0, np.float32)
    c0 = np.asarray(c0, np.float32)
    embed_w = np.asarray(embed_w, np.float32)
    w_ih = np.asarray(w_ih, np.float32)
    w_hh = np.asarray(w_hh, np.float32)
    b_ih = np.asarray(b_ih, np.float32)
    b_hh = np.asarray(b_hh, np.float32)
    lin_w = np.asarray(lin_w, np.float32)
    lin_b = np.asarray(lin_b, np.float32)
    maxlen = int(maxlen)
    batch = captions.shape[0]

    # Sort rows by descending length (stable). pack_padded_sequence requires
    # descending lengths, so perm is normally the identity; the permutation
    # fallback keeps us correct on arbitrary length order.
    ln = lengths.astype(np.int64)
    perm = np.argsort(-ln, kind="stable")
    identity_perm = bool(np.all(perm == np.arange(batch)))
    lns = ln[perm]

    n_ts = []
    for t in range(maxlen):
        n = int((lns > t).sum())
        if n == 0:
            break
        n_ts.append(n)
    L = int(sum(n_ts))
    offs = np.concatenate([[0], np.cumsum(n_ts)]).astype(int)

    # host: x-part of the gates for all packed rows. Teacher-forced inputs
    # are embedding rows -> lookup into embed_w[used] @ w_ih.T (small BLAS).
    bias2 = (b_ih + b_hh).astype(np.float32)
    wihT = np.ascontiguousarray(w_ih.T)
    xW = np.empty((L, G4), np.float32)
    n0 = int(n_ts[0])
    xW[:n0] = features[perm[:n0]] @ wihT
    if L > n0:
        toks = np.concatenate(
            [captions[perm[: int(n)], t - 1] for t, n in enumerate(n_ts) if t > 0]
        )
        uniq, inv = np.unique(toks, return_inverse=True)
        eW = embed_w[uniq] @ wihT
        xW[n0:] = eW[inv]
    xW += bias2
    xWT = np.ascontiguousarray(xW.T).astype(BF)
    # dense [p, m*n0+l] arrangement of the step-0 slice for the head DMA
    xWh = np.ascontiguousarray(
        xW[:n0].reshape(n0, NM, P).transpose(2, 1, 0).reshape(P, NM * n0)
    ).astype(BF)

    whhT = np.ascontiguousarray(w_hh.T).astype(BF)
    linT = np.ascontiguousarray(lin_w.T).astype(BF)
    zero_init = not (np.any(h0) or np.any(c0))
    use_linb = bool(np.any(lin_b))

    nc = _get_program(n_ts, zero_init, use_linb)

    in_maps = []
    for s in range(NCORES):
        m = {
            "xWT": xWT,
            "xWh": xWh,
            "whh": whhT,
            "linT": np.ascontiguousarray(linT[:, VS * s : VS * (s + 1)]),
        }
        if not zero_init:
            m["h0T"] = np.ascontiguousarray(h0[perm].T).astype(BF)
            m["c0T"] = np.ascontiguousarray(c0[perm].T)
        if use_linb:
            m["linb"] = np.ascontiguousarray(
                lin_b[VS * s : VS * (s + 1)].reshape(1, VS).astype(BF)
            )
        in_maps.append(m)

    res = run_bass_kernel_spmd(
        nc, in_maps, core_ids=list(range(NCORES)), trace=_trace
    )
    out = np.concatenate(
        [np.asarray(r["out"]).astype(np.float32) for r in res.results], axis=1
    )

    if not identity_perm:
        # map packed rows computed in sorted order back to original order
        src = np.empty(L, np.int64)
        pos = 0
        inv_pos = {}
        for t, n in enumerate(n_ts):
            for j in range(n):
                inv_pos[(t, int(perm[j]))] = offs[t] + j
        for t in range(maxlen):
            for i in np.nonzero(ln > t)[0]:
                src[pos] = inv_pos[(t, int(i))]
                pos += 1
        out = out[src]

    if _trace:
        return out, res
    return out
